# revision 64
# baseline (speedup 1.0000x reference)
"""DeepseekV2 MoE layer on 8 Trainium2 NeuronCores — fp8 DoubleRow edition.

Strategy (expert-parallel, per the sharding hint):
  - Router gate + grouped top-k computed on host (0.03% of module FLOPs);
    it determines the dispatch, which IS the input sharding.
  - 16 routed experts paired big-count-with-small-count onto 8 cores
    (2 experts per core, token lists gathered host-side, padded to a
    shared per-slot capacity so all cores run one SPMD program).
  - Shared-expert MLP is data-parallel over tokens: each core runs
    T/8 = 512 tokens through the full shared MLP.
  - All matmuls use fp8(e4m3) in DoubleRow perf mode (0.5 PE cycles per
    output column vs 1.0 for bf16).  Accuracy is preserved with a
    3-product hi/lo split: every operand A is stored as A_hi = q(A*s)
    plus residual A_lo = q(A*s - A_hi) at the SAME scale, and each
    128-contraction chunk pair is covered by 3 DoubleRow matmuls:
      A: (w_hi k0)(x_hi k0) + (w_hi k1)(x_hi k1)     [both hi products]
      B: (w_hi k0)(x_lo k0) + (w_lo k0)(x_hi k0)     [cross terms k0]
      C: (w_hi k1)(x_lo k1) + (w_lo k1)(x_hi k1)     [cross terms k1]
    dropping only the O(eps^2) lo*lo term -> ~1e-3 rel error, at 0.75x
    the PE cycles of bf16 (1.5 cycles per column per 256 contraction).
  - f32 PSUM accumulation; fp16 outputs (device applies routing weights
    and all descales); host transposes/combines during unshard.
"""

import sys

sys.path.insert(0, "/opt/trn_rl_repo")

import copy

import ml_dtypes
import numpy as np

import concourse.bass as bass
import concourse.mybir as mybir
import concourse.tile as tile
from concourse.alu_op_type import AluOpType
from concourse.bass_utils import run_bass_kernel_spmd

DT = mybir.dt
E4 = ml_dtypes.float8_e4m3

T, D, E, I = 4096, 2048, 16, 1024
TOP_K, N_GROUP, TOPK_GROUP = 4, 4, 2
ROUTED_SCALE = 2.5
SHARED_I = 2048
N_CORES = 8
P = 128
NCHUNK = 512  # token chunk (matmul moving free dim; one PSUM bank of f32)
ND = D // P  # 16 contraction chunks over D

S_X = 16.0  # x scale (2^4)
S_W = 128.0  # weight scale (2^7)
INV_GU = 1.0 / (S_X * S_W)  # 2^-11: descale for mm1 psum
S_HC = 2.0 ** -8  # h_sc = sg * u_psum * 2^-8 = h_true * 8
INV_Y = 2.0 ** -10  # mm2 psum = y_true * 8 * 128


# ---------------------------------------------------------------- wait split
def _split_excess_waits(nc, limit=1):
    """This walrus build rejects >1 sync-wait command per instruction.
    Move excess waits onto fresh same-engine NOPs inserted just before."""
    template = bass.Bass(target_bir_lowering=False).sync.nop(nofuse=True).ins
    ctr = 0
    for bb in nc.main_func.blocks:
        out = []
        changed = False
        for ins in bb.instructions:
            si = ins.sync_info
            if si is not None and si.on_wait and len(si.on_wait) > limit:
                waits = list(si.on_wait)
                for w in waits[:-limit]:
                    ctr += 1
                    nop = copy.deepcopy(template)
                    nop.name = f"I-wsplit-{ctr}"
                    nop.engine = ins.engine
                    nop.bass_nofuse = True
                    nop.sync_info = mybir.SyncInfo(on_wait=[w], on_update=[])
                    nc.register_instruction(nop, overwrite=True)
                    out.append(nop)
                ins.sync_info = mybir.SyncInfo(
                    on_wait=waits[-limit:], on_update=list(si.on_update)
                )
                changed = True
            out.append(ins)
        if changed:
            bb.instructions = out
    return ctr


# ---------------------------------------------------------------- routing
def _gate_logits(x, gate_w):
    # Match the reference's jax-f32 CPU matmul as closely as possible.
    try:
        import jax
        import jax.numpy as jnp

        cpu = jax.devices("cpu")[0]
        with jax.default_device(cpu):
            return np.asarray(jnp.matmul(jnp.asarray(x), jnp.asarray(gate_w)))
    except Exception:
        return (x @ gate_w).astype(np.float32)


def _route(x, gate_w, e_bias):
    logits = _gate_logits(x, gate_w)  # [T, E] f32
    scores = (1.0 / (1.0 + np.exp(-logits))).astype(np.float32)
    sfc = scores + e_bias[None, :]
    grp = sfc.reshape(T, N_GROUP, E // N_GROUP)
    group_scores = np.sort(grp, axis=-1)[:, :, -2:].sum(-1)  # [T, G]
    group_idx = np.argsort(-group_scores, axis=-1, kind="stable")[:, :TOPK_GROUP]
    group_mask = np.zeros((T, N_GROUP), bool)
    group_mask[np.arange(T)[:, None], group_idx] = True
    expert_mask = np.repeat(group_mask, E // N_GROUP, axis=1)
    masked = np.where(expert_mask, sfc, -np.inf)
    topk_idx = np.argsort(-masked, axis=-1, kind="stable")[:, :TOP_K]  # [T, 4]
    topk_w = np.take_along_axis(scores, topk_idx, axis=1)
    topk_w = topk_w / topk_w.sum(axis=1, keepdims=True)
    return topk_idx.astype(np.int64), topk_w.astype(np.float32)


# ---------------------------------------------------------------- program
_PROGRAM_CACHE = {}

DRM = None  # set lazily (mybir.MatmulPerfMode.DoubleRow)


def _emit_mm_pair(nc, ps, w_t, x_t, pr, sz, off, start, stop):
    """3 DoubleRow matmuls covering contraction-chunk pair (2pr, 2pr+1).

    w_t slices: [:, k, 0, :] = hi, [:, k, 1, :] = lo   (stationary)
    x_t slices: [:, k, 0, c] = lo, [:, k, 1, c] = hi   (moving)
    """
    k0, k1 = 2 * pr, 2 * pr + 1
    nc.tensor.matmul(
        ps[:, :sz], w_t[:, k0:k0 + 2, 0, :], x_t[:, k0:k0 + 2, 1, off:off + sz],
        start=start, stop=False, perf_mode=DRM)
    nc.tensor.matmul(
        ps[:, :sz], w_t[:, k0, 0:2, :], x_t[:, k0, 0:2, off:off + sz],
        start=False, stop=False, perf_mode=DRM)
    nc.tensor.matmul(
        ps[:, :sz], w_t[:, k1, 0:2, :], x_t[:, k1, 0:2, off:off + sz],
        start=False, stop=stop, perf_mode=DRM)


def _emit_mm_pair2(nc, ps, w_t, x_t, pr, sz, off, start, stop):
    """2-product variant: full-W x X_hi (X residual dropped). Used on half
    the routed down-projection contraction; costs 1.0 instead of 1.5
    PE-cycles per column per 256-contraction, adds ~1.2e-2 quantization
    noise (budget 2e-2, base scheme uses 3.4e-3)."""
    k0 = 2 * pr
    nc.tensor.matmul(
        ps[:, :sz], w_t[:, k0:k0 + 2, 0, :], x_t[:, k0:k0 + 2, 1, off:off + sz],
        start=start, stop=False, perf_mode=DRM)
    nc.tensor.matmul(
        ps[:, :sz], w_t[:, k0:k0 + 2, 1, :], x_t[:, k0:k0 + 2, 1, off:off + sz],
        start=False, stop=stop, perf_mode=DRM)


DROP_PAIRS = 3  # routed-mm2 contraction pairs using the 2-product variant


def _emit_mm_group(nc, ps, w_t, x_t, n_k, sz, off, drop=0):
    # 2-product pairs sit at the END of the contraction so the last-produced
    # h chunks need no residual (shorter consumer chain into mm2)
    npairs = n_k // 2
    for pr in range(npairs):
        emit = _emit_mm_pair2 if pr >= npairs - drop else _emit_mm_pair
        emit(nc, ps, w_t, x_t, pr, sz, off, pr == 0, pr == npairs - 1)


def _emit_expert(nc, tc, pools, xt_h, w1_h, w2_h, wr_h, y_h, C, twoI, apply_wr,
                 streamed_start=False, small_flush=False, filler=None):
    n_i = twoI // P  # mm1 output chunks (gate/up interleaved in packed order)
    n_h = n_i // 2  # h chunks (= I/128)
    chunks = [(o, min(NCHUNK, C - o)) for o in range(0, C, NCHUNK)]

    (x_pool, w1_pool, w2_pool, sg_pool, hsc_pool, h_pool, y_pool, wr_pool,
     ps_gu, ps_dn) = pools

    # whole-expert x resident tile [P, ND, 2, C]; split the load and
    # interleave the first w1 group DMAs so the DMA pipe feeds the first
    # matmul group as early as possible (subtile deps let each k-pair
    # start as soon as its x piece + w1 group have landed)
    x_t = x_pool.tile([P, ND, 2, C], DT.float8e4, name="xt")
    w1_pre = {}
    w1_pre[0] = w1_pool.tile([P, 2, ND, 2, P], DT.float8e4, name="w1t")
    w1_pre[1] = w1_pool.tile([P, 2, ND, 2, P], DT.float8e4, name="w1t")
    if streamed_start:
        # finest-grained opening: first matmul needs only w1[mi0] + x piece 0
        nc.sync.dma_start(w1_pre[0][:, 0], w1_h[:, 0, :, :, :])
        nc.sync.dma_start(x_t[:, 0:2, :, :], xt_h[:, 0:2, :, :])
        nc.sync.dma_start(w1_pre[0][:, 1], w1_h[:, 1, :, :, :])
    else:
        nc.sync.dma_start(w1_pre[0][:], w1_h[:, 0:2, :, :, :])
        nc.sync.dma_start(x_t[:, 0:2, :, :], xt_h[:, 0:2, :, :])
    for q in range(1, 8):
        nc.sync.dma_start(x_t[:, 2 * q:2 * q + 2, :, :], xt_h[:, 2 * q:2 * q + 2, :, :])
        if q == 3:
            nc.sync.dma_start(w1_pre[1][:, 0], w1_h[:, 2, :, :, :])
        elif q == 5:
            nc.sync.dma_start(w1_pre[1][:, 1], w1_h[:, 3, :, :, :])

    wr_t = None
    if apply_wr:
        wr_t = wr_pool.tile([P, C], DT.float16, name="wr")
        nc.sync.dma_start(wr_t[:], wr_h[:, :])

    h_t = h_pool.tile([P, n_h, 2, C], DT.float8e4, name="ht")

    # ---- mm1: packed-mi order is (gate hc, up hc) interleaved
    sig = mybir.ActivationFunctionType.Sigmoid
    cop = mybir.ActivationFunctionType.Copy
    w1_t = None
    sg_tiles = {}
    start_ps = {}
    n_il = 2
    if streamed_start:
        # While x streams in, interleave the first mi-groups' matmuls
        # piece-by-piece across all N-chunks so the in-order PE never waits
        # on an x piece while other ready work exists.
        for mi in range(n_il):
            for ci in range(len(chunks)):
                start_ps[(mi, ci)] = ps_gu.tile([P, NCHUNK], DT.float32,
                                                name="psg")
        for pr in range(ND // 2):
            for mi in range(n_il):
                for ci, (off, sz) in enumerate(chunks):
                    _emit_mm_pair(nc, start_ps[(mi, ci)], w1_pre[mi // 2][:, mi % 2],
                                  x_t, pr, sz, off, pr == 0, pr == ND // 2 - 1)
            if filler is not None and pr < ND // 2 - 1:
                # fill the x-stream pacing deficit so the in-order PE stays
                # continuously busy (keeps the p-state ramp alive too)
                wu_w, wu_x, wu_ps = filler
                for _ in range(3):
                    nc.tensor.matmul(wu_ps[:, :P], wu_w[:], wu_x[:], start=True,
                                     stop=True, perf_mode=DRM)
    for mi in range(n_i):
        if mi % 2 == 0:
            if mi // 2 in w1_pre:
                w1_t = w1_pre.pop(mi // 2)
            else:
                w1_t = w1_pool.tile([P, 2, ND, 2, P], DT.float8e4, name="w1t")
                nc.sync.dma_start(w1_t[:], w1_h[:, mi:mi + 2, :, :, :])
        hc = mi // 2
        is_up = mi % 2 == 1
        for ci, (off, sz) in enumerate(chunks):
            if streamed_start and mi < n_il:
                ps = start_ps[(mi, ci)]
            else:
                ps = ps_gu.tile([P, NCHUNK], DT.float32, name="psg")
                _emit_mm_group(nc, ps, w1_t[:, mi % 2], x_t, ND, sz, off)
            if not is_up:
                # sigmoid(g) in bf16; silu assembled on DVE in the up pass
                sgm = sg_pool.tile([P, NCHUNK], DT.bfloat16, name="sgm")
                nc.scalar.activation(sgm[:, :sz], ps[:, :sz], sig, scale=INV_GU)
                # sgs = (g_psum * 2^-19) * sigmoid(g) = silu(g) * 2^-8, bf16
                sgs = sg_pool.tile([P, NCHUNK], DT.bfloat16, name="sgs")
                nc.vector.scalar_tensor_tensor(
                    sgs[:, :sz], ps[:, :sz], INV_GU * S_HC, sgm[:, :sz],
                    AluOpType.mult, AluOpType.mult)
                sg_tiles[ci] = sgs
            else:
                hsc = hsc_pool.tile([P, NCHUNK], DT.bfloat16, name="hsc")
                nc.vector.tensor_mul(hsc[:, :sz], sg_tiles[ci][:, :sz], ps[:, :sz])
                nc.scalar.activation(
                    h_t[:, hc, 1, off:off + sz], hsc[:, :sz], cop)
                if not (apply_wr and hc >= n_h - 2 * DROP_PAIRS):
                    # h residual unused by the 2-product half of routed mm2
                    nc.vector.tensor_sub(
                        h_t[:, hc, 0, off:off + sz], hsc[:, :sz],
                        h_t[:, hc, 1, off:off + sz])

    # ---- mm2: d2-outer, ci-inner; y tiles flushed in groups of d2
    if not apply_wr:
        flushes = [2, 2, 2, 2, 2, 2, 2, 1, 1] if small_flush else [2] * (ND // 2)
    elif small_flush:
        flushes = [2, 2, 2, 2, 2, 2, 2, 1, 1]  # tapered: shortens the final drain
    else:
        flushes = [4, 4, 4, 4]
    f_start = {}
    acc = 0
    for f in flushes:
        for d2 in range(acc, acc + f):
            f_start[d2] = (acc, f)
        acc += f
    y_view = y_h.rearrange("(g p) t -> p g t", p=P)
    y_tiles = {}
    w2_t = None
    for d2 in range(ND):
        if d2 % 2 == 0:
            w2_t = w2_pool.tile([P, 2, n_h, 2, P], DT.float8e4, name="w2t")
            nc.sync.dma_start(w2_t[:], w2_h[:, d2:d2 + 2, :, :, :])
        g0, fl = f_start[d2]
        for ci, (off, sz) in enumerate(chunks):
            if d2 == g0:
                y_tiles[ci] = y_pool.tile([P, fl, NCHUNK], DT.float16, name="yt")
            ps = ps_dn.tile([P, NCHUNK], DT.float32, name="psd")
            _emit_mm_group(nc, ps, w2_t[:, d2 % 2], h_t, n_h, sz, off,
                           drop=DROP_PAIRS if apply_wr else 0)
            if apply_wr:
                nc.vector.tensor_mul(
                    y_tiles[ci][:, d2 - g0, :sz], ps[:, :sz], wr_t[:, off:off + sz])
            else:
                nc.scalar.activation(
                    y_tiles[ci][:, d2 - g0, :sz], ps[:, :sz], cop, scale=INV_Y)
            if d2 == g0 + fl - 1:
                # output DMAs go on the ACT hwdge queue so SP's in-order
                # issue never blocks the next expert's input loads
                nc.scalar.dma_start(
                    y_view[:, g0:d2 + 1, off:off + sz], y_tiles[ci][:, :, :sz])


def _build_program(C1, C2):
    global DRM
    DRM = mybir.MatmulPerfMode.DoubleRow
    key = (C1, C2)
    if key in _PROGRAM_CACHE:
        return _PROGRAM_CACHE[key]

    nc = bass.Bass(target_bir_lowering=False)
    TS = T // N_CORES  # shared tokens per core

    n_i_r, n_h_r = 2 * I // P, I // P
    n_i_s, n_h_s = 2 * SHARED_I // P, SHARED_I // P

    xt1 = nc.dram_tensor("xt1", [P, ND, 2, C1], DT.float8e4, kind="ExternalInput")
    xt2 = nc.dram_tensor("xt2", [P, ND, 2, C2], DT.float8e4, kind="ExternalInput")
    xts = nc.dram_tensor("xts", [P, ND, 2, TS], DT.float8e4, kind="ExternalInput")
    w1a = nc.dram_tensor("w1a", [P, n_i_r, ND, 2, P], DT.float8e4, kind="ExternalInput")
    w2a = nc.dram_tensor("w2a", [P, ND, n_h_r, 2, P], DT.float8e4, kind="ExternalInput")
    w1b = nc.dram_tensor("w1b", [P, n_i_r, ND, 2, P], DT.float8e4, kind="ExternalInput")
    w2b = nc.dram_tensor("w2b", [P, ND, n_h_r, 2, P], DT.float8e4, kind="ExternalInput")
    ws1 = nc.dram_tensor("ws1", [P, n_i_s, ND, 2, P], DT.float8e4, kind="ExternalInput")
    ws2 = nc.dram_tensor("ws2", [P, ND, n_h_s, 2, P], DT.float8e4, kind="ExternalInput")
    wr1 = nc.dram_tensor("wr1", [P, C1], DT.float16, kind="ExternalInput")
    wr2 = nc.dram_tensor("wr2", [P, C2], DT.float16, kind="ExternalInput")
    y1 = nc.dram_tensor("y1", [D, C1], DT.float16, kind="ExternalOutput")
    y2 = nc.dram_tensor("y2", [D, C2], DT.float16, kind="ExternalOutput")
    ys = nc.dram_tensor("ys", [D, TS], DT.float16, kind="ExternalOutput")

    with tile.TileContext(nc) as tc:
        with (
            tc.tile_pool(name="xp", bufs=2) as x_pool,
            tc.tile_pool(name="w1p", bufs=3) as w1_pool,
            tc.tile_pool(name="w2p", bufs=2) as w2_pool,
            tc.tile_pool(name="sgp", bufs=4) as sg_pool,
            tc.tile_pool(name="hscp", bufs=2) as hsc_pool,
            tc.tile_pool(name="hp", bufs=2) as h_pool,
            tc.tile_pool(name="yp", bufs=6) as y_pool,
            tc.tile_pool(name="wrp", bufs=2) as wr_pool,
            tc.tile_pool(name="psgu", bufs=6, space="PSUM") as ps_gu,
            tc.tile_pool(name="psdn", bufs=2, space="PSUM") as ps_dn,
        ):
            pools = (x_pool, w1_pool, w2_pool, sg_pool, hsc_pool, h_pool,
                     y_pool, wr_pool, ps_gu, ps_dn)
            # PE warmup: dummy DoubleRow matmuls on zeroed scratch during the
            # initial DMA fill, so the tensor engine p-state is fully ramped
            # (2.4 GHz needs 3us of continuous busy) when real work arrives.
            wu_w = sg_pool.tile([P, 2, P], DT.float8e4, name="wuw", bufs=1)
            wu_x = sg_pool.tile([P, 2, P], DT.float8e4, name="wux", bufs=1)
            nc.vector.memset(wu_w[:], 0.0)
            nc.gpsimd.memset(wu_x[:], 0.0)
            wu_ps = ps_dn.tile([P, NCHUNK], DT.float32, name="psd")
            for _ in range(80):
                nc.tensor.matmul(wu_ps[:, :P], wu_w[:], wu_x[:], start=True,
                                 stop=True, perf_mode=DRM)
            _emit_expert(nc, tc, pools, xts, ws1, ws2, None, ys, TS, 2 * SHARED_I,
                         False, streamed_start=True, filler=(wu_w, wu_x, wu_ps))
            _emit_expert(nc, tc, pools, xt1, w1a, w2a, wr1, y1, C1, 2 * I, True)
            _emit_expert(nc, tc, pools, xt2, w1b, w2b, wr2, y2, C2, 2 * I, True,
                         small_flush=True)

    _split_excess_waits(nc, limit=1)
    _PROGRAM_CACHE[key] = nc
    return nc


# ---------------------------------------------------------------- packing
def _hilo(a, s):
    hi = (a * s).astype(E4)
    lo = ((a * s).astype(np.float32) - hi.astype(np.float32)).astype(E4)
    return hi, lo


def _pack_w(W, interleave_gate_up):
    """W [K, M] f32 -> [P, M/P, K/P, 2, P] e4m3 with (hi, lo) pairs.
    If interleave_gate_up, M-chunk order is (g0, u0, g1, u1, ...)."""
    K, M = W.shape
    kc, mi = K // P, M // P
    if interleave_gate_up:
        nh = mi // 2
        order = np.stack([np.arange(nh), np.arange(nh) + nh], 1).ravel()
        W = W.reshape(K, mi, P)[:, order, :].reshape(K, M)
    hi, lo = _hilo(W, S_W)
    A = np.stack([hi, lo], axis=0)  # [2, K, M]
    A = A.reshape(2, kc, P, mi, P).transpose(2, 3, 1, 0, 4)  # [p, mi, kc, 2, m]
    return np.ascontiguousarray(A)


def _pack_x(xhi_g, xlo_g):
    """Gathered hi/lo [D, C] e4m3 -> [P, ND, 2, C] with (lo, hi) pairs."""
    C = xhi_g.shape[1]
    A = np.stack([xlo_g, xhi_g], axis=0)  # [2, D, C]
    A = A.reshape(2, ND, P, C).transpose(2, 1, 0, 3)  # [p, kc, 2, c]
    return np.ascontiguousarray(A)


def _cap(n):
    return max(P, int(n))


# ---------------------------------------------------------------- kernel
def _prepare(hidden_states, gate_w, e_bias, w_gate_up, w_down, ws_gate_up, ws_down):
    x = np.asarray(hidden_states, dtype=np.float32)
    topk_idx, topk_w = _route(x, np.asarray(gate_w), np.asarray(e_bias))

    # dispatch: token lists per expert, sorted-stable by expert id
    flat_e = topk_idx.ravel()
    order = np.argsort(flat_e, kind="stable")
    pair_tok = order // TOP_K
    pair_w = (topk_w.ravel()[order] * (ROUTED_SCALE * INV_Y)).astype(np.float32)
    counts = np.bincount(flat_e, minlength=E)
    starts = np.zeros(E + 1, np.int64)
    np.cumsum(counts, out=starts[1:])

    # expert -> core assignment: pair largest with smallest
    by_count = np.argsort(-counts, kind="stable")
    slotA = by_count[:N_CORES]
    slotB = by_count[E - 1:N_CORES - 1:-1]  # reversed smallest half
    C1 = _cap(counts[slotA].max())
    C2 = _cap(counts[slotB].max())

    nc = _build_program(C1, C2)

    xT = np.ascontiguousarray(x.T)  # [D, T]
    xhi, xlo = _hilo(xT, S_X)

    ws1_p = _pack_w(np.asarray(ws_gate_up), True)
    ws2_p = _pack_w(np.asarray(ws_down), False)
    w_gate_up = np.asarray(w_gate_up)
    w_down = np.asarray(w_down)

    TS = T // N_CORES
    in_maps = []
    core_info = []
    for c in range(N_CORES):
        eA, eB = int(slotA[c]), int(slotB[c])
        m = {}
        info = []
        for e_id, C, xt_name, wr_name in (
            (eA, C1, "xt1", "wr1"),
            (eB, C2, "xt2", "wr2"),
        ):
            idx = pair_tok[starts[e_id]:starts[e_id + 1]]
            w = pair_w[starts[e_id]:starts[e_id + 1]]
            n_e = len(idx)
            idx_pad = np.zeros(C, np.int64)
            idx_pad[:n_e] = idx
            w_pad = np.zeros(C, np.float16)
            w_pad[:n_e] = w
            m[xt_name] = _pack_x(xhi[:, idx_pad], xlo[:, idx_pad])
            m[wr_name] = np.ascontiguousarray(np.broadcast_to(w_pad, (P, C)))
            info.append((idx, n_e))
        m["xts"] = _pack_x(xhi[:, c * TS:(c + 1) * TS], xlo[:, c * TS:(c + 1) * TS])
        m["w1a"] = _pack_w(w_gate_up[eA], True)
        m["w2a"] = _pack_w(w_down[eA], False)
        m["w1b"] = _pack_w(w_gate_up[eB], True)
        m["w2b"] = _pack_w(w_down[eB], False)
        m["ws1"] = ws1_p
        m["ws2"] = ws2_p
        in_maps.append(m)
        core_info.append(info)
    return nc, in_maps, core_info


def _combine(res_results, core_info):
    TS = T // N_CORES
    out = np.zeros((T, D), np.float32)
    for c in range(N_CORES):
        (idxA, nA), (idxB, nB) = core_info[c]
        out[idxA] += res_results[c]["y1"][:, :nA].T.astype(np.float32)
        out[idxB] += res_results[c]["y2"][:, :nB].T.astype(np.float32)
        out[c * TS:(c + 1) * TS] += res_results[c]["ys"].T.astype(np.float32)
    return out


def kernel(hidden_states, gate_w, e_bias, w_gate_up, w_down, ws_gate_up, ws_down):
    nc, in_maps, core_info = _prepare(
        hidden_states, gate_w, e_bias, w_gate_up, w_down, ws_gate_up, ws_down
    )
    res = run_bass_kernel_spmd(nc, in_maps, list(range(N_CORES)))
    return _combine(res.results, core_info)
